# revision 16
# baseline (speedup 1.0000x reference)
"""Trainium2 Bass kernel for TernaryLinear: y[b,m,n] = sum_k x[b,m,k] * w[k,n].

Shapes: x (4, 2048, 4096) fp32, w (4096, 4096) ternary fp32 -> y (4, 2048, 4096).

Strategy: flatten x to 8192 rows, row-shard across 8 NeuronCores (1024 rows
each), replicate w. All matmuls run in fp8e4 (e4m3) DoubleRow mode, which
contracts 256 k-values per instruction at the same 1-column/cycle stream rate
as bf16 -- 2x the MAC throughput. Ternary {-1,0,1} w is exact in e4m3; x is
not (e4m3 alone gives 2.65% rel err vs the 2% gate), so x is split into
x_hi = e4m3(x) and x_lo = e4m3(x - x_hi) and the first NLO of 16 k-granules
(256 k each) get a second compensation matmul with x_lo, bringing the error
to ~1.9e-2 < 2e-2 while costing only (16+NLO)/32 of the bf16 PE time.

Per core: x_hi (16 granules) + x_lo (NLO granules) resident in SBUF as
[128kp x 8mt x 2slot x 128m] e4m3 tiles (each [:, mt] slice is a contiguous
256B-per-partition DoubleRow stationary operand -- strided slices cost ~45ns
per LDWEIGHTS). n-chunk 0 runs granule-major so the PE starts after one
granule of x + one w tile; its w tiles stream per-granule while x preloads on
the other DMA queue, and the two tiles the first matmul waits on (w0, mt=0
slice of xh0) go first on separate queues. A dozen dummy matmuls on memset
tiles warm the PE p-states while those DMAs land. n-chunks 1..7 run mt-outer
chains (24 matmuls into one PSUM bank, then evict) so evictions stagger and
the next chunk's start=True matmuls never wait on bank-free; their w arrives
as one 2MB tile per chunk, double-buffered. Evictions run on the vector
engine (fp32 PSUM -> fp16 SBUF), outputs DMA out on alternating queues and
the host upcasts to fp32. No cross-core communication; host concatenates the
row shards.

Measured on 8xTRN2 (axon): 347.7-348.4us vs the 460.6us fp16 baseline
(1.32x), rel err 1.876e-02 (gate 2e-2), PE stream at the 1-column/cycle
limit: 1536 DoubleRow matmuls x ~216.5ns.
"""

import sys

for _p in ("/opt/trn_rl_repo", "/opt/pypackages"):
    if _p not in sys.path:
        sys.path.append(_p)

import ml_dtypes
import numpy as np

import concourse.bass as bass
import concourse.bacc as bacc
import concourse.mybir as mybir
import concourse.tile as tile
from concourse.bass_utils import run_bass_kernel_spmd

P = 128
NCORES = 8
B, M, K, N = 4, 2048, 4096, 4096
R = B * M            # 8192 rows total
MR = R // NCORES     # 1024 rows per core
G = K // (2 * P)     # 16 k-granules of 256 (one DoubleRow contraction each)
NLO = 8              # leading granules that get the x_lo compensation pass
MT = MR // P         # 8 m-tiles per core
NCH = 512            # moving free dim per matmul (one PSUM bank of fp32)
NCHUNKS = N // NCH   # 8
F32 = mybir.dt.float32
F16 = mybir.dt.float16
F8 = mybir.dt.float8e4
DR = mybir.MatmulPerfMode.DoubleRow

_PROGRAM = None


def _build_program():
    nc = bacc.Bacc(
        "TRN2",
        target_bir_lowering=False,
        debug=False,
        num_devices=NCORES,
    )
    xh = nc.dram_tensor("xh", [G, P, MT, 2, P], F8, kind="ExternalInput").ap()
    xl = nc.dram_tensor("xl", [NLO, P, MT, 2, P], F8, kind="ExternalInput").ap()
    w = nc.dram_tensor("w", [NCHUNKS, P, G, 2, NCH], F8, kind="ExternalInput").ap()
    y = nc.dram_tensor("y", [MT, P, N], F16, kind="ExternalOutput").ap()

    with tile.TileContext(nc) as tc:
        with (
            tc.tile_pool(name="xres", bufs=1) as xpool,
            tc.tile_pool(name="wstream", bufs=10) as wpool,
            tc.tile_pool(name="wbig", bufs=2) as wbpool,
            tc.tile_pool(name="outstage", bufs=8) as opool,
            tc.tile_pool(name="acc", bufs=8, space="PSUM") as ppool,
        ):
            xhtiles = [None] * G
            xltiles = [None] * NLO
            xh0a = xh0b = None

            def xsliceH(g, mt):
                if g == 0:
                    return xh0a[:] if mt == 0 else xh0b[:, mt - 1]
                return xhtiles[g][:, mt]

            def evict(nch, mt, ps):
                # All evictions on the vector engine: scalar.copy would pull
                # in a 1.3us ACT_TABLE_LOAD that delays the scalar DMA queue
                # at startup, and DVE has plenty of slack.
                ot = opool.tile([P, NCH], F16, tag="o", name=f"o{nch}_{mt}")
                nc.vector.tensor_copy(ot[:], ps[:])
                dma_eng = nc.scalar if mt % 2 == 0 else nc.sync
                dma_eng.dma_start(out=y[mt, :, bass.ts(nch, NCH)], in_=ot[:])

            # --- PE p-state warmup: ~12 dummy DoubleRow matmuls on memset
            # tiles fill the PE pipeline while the first x/w DMAs land, so the
            # real stream starts near full clock instead of ramping through
            # the slow p-states on real work.
            warm_x = xpool.tile([P, 2, 16], F8, tag="warm_x", name="warm_x")
            warm_w = xpool.tile([P, 2, 256], F8, tag="warm_w", name="warm_w")
            nc.any.memset(warm_x, 0)
            nc.any.memset(warm_w, 0)
            warm_ps = ppool.tile([P, NCH], F32, tag="acc", name="warm_ps")
            for i in range(12):
                nc.tensor.matmul(
                    out=warm_ps[:16, :256],
                    lhsT=warm_x[:],
                    rhs=warm_w[:],
                    start=True,
                    stop=True,
                    perf_mode=DR,
                )

            # --- n-chunk 0: granule-major, x preload interleaved. Per granule
            # the sync queue carries xh (+ xl for odd granules) and the scalar
            # queue carries the w tile first (the first matmul waits on it),
            # then xl for even granules. Granule 0's xh is split so the first
            # matmul only waits on the 32KB mt=0 slice.
            psums = [
                ppool.tile([P, NCH], F32, tag="acc", name=f"ps0_{mt}")
                for mt in range(MT)
            ]
            for g in range(G):
                wt = wpool.tile([P, 2, NCH], F8, tag="w", name=f"w0_{g}")
                if g == 0:
                    # The first matmul waits on w0 + the mt=0 slice of xh0;
                    # put them first on SEPARATE queues (cold-queue transfers
                    # run ~50GB/s, so serializing them costs ~2us).
                    nc.scalar.dma_start(out=wt[:], in_=w[0, :, g])
                    xh0a = xpool.tile([P, 2, P], F8, tag="xh0a", name="xh0a")
                    nc.sync.dma_start(out=xh0a[:], in_=xh[0, :, 0])
                    xh0b = xpool.tile(
                        [P, MT - 1, 2, P], F8, tag="xh0b", name="xh0b"
                    )
                    nc.sync.dma_start(out=xh0b[:], in_=xh[0, :, 1:])
                else:
                    xt = xpool.tile(
                        [P, MT, 2, P], F8, tag=f"xh{g}", name=f"xh{g}"
                    )
                    nc.sync.dma_start(out=xt[:], in_=xh[g])
                    xhtiles[g] = xt
                    nc.scalar.dma_start(out=wt[:], in_=w[0, :, g])
                if g < NLO:
                    lt = xpool.tile(
                        [P, MT, 2, P], F8, tag=f"xl{g}", name=f"xl{g}"
                    )
                    (nc.sync if g % 2 else nc.scalar).dma_start(
                        out=lt[:], in_=xl[g]
                    )
                    xltiles[g] = lt
                for mt in range(MT):
                    nc.tensor.matmul(
                        out=psums[mt][:],
                        lhsT=xsliceH(g, mt),
                        rhs=wt[:],
                        start=(g == 0),
                        stop=(g == G - 1),
                        perf_mode=DR,
                    )
                if g < NLO:
                    for mt in range(MT):
                        nc.tensor.matmul(
                            out=psums[mt][:],
                            lhsT=xltiles[g][:, mt],
                            rhs=wt[:],
                            start=False,
                            stop=False,
                            perf_mode=DR,
                        )
            for mt in range(MT):
                evict(0, mt, psums[mt])

            # --- n-chunks 1..7: mt-outer chains; per-chunk w arrives as one
            # 2MB tile, double-buffered on the sync queue.
            for nch in range(1, NCHUNKS):
                wb = wbpool.tile([P, G, 2, NCH], F8, tag="wb", name=f"wb{nch}")
                nc.sync.dma_start(out=wb[:], in_=w[nch])
                for mt in range(MT):
                    ps = ppool.tile(
                        [P, NCH], F32, tag="acc", name=f"ps{nch}_{mt}"
                    )
                    for g in range(G):
                        nc.tensor.matmul(
                            out=ps[:],
                            lhsT=xsliceH(g, mt),
                            rhs=wb[:, g],
                            start=(g == 0),
                            stop=False,
                            perf_mode=DR,
                        )
                    for g in range(NLO):
                        nc.tensor.matmul(
                            out=ps[:],
                            lhsT=xltiles[g][:, mt],
                            rhs=wb[:, g],
                            start=False,
                            stop=(g == NLO - 1),
                            perf_mode=DR,
                        )
                    evict(nch, mt, ps)
    nc.compile()
    return nc


def _get_program():
    global _PROGRAM
    if _PROGRAM is None:
        _PROGRAM = _build_program()
    return _PROGRAM


def _prepare_in_maps(x: np.ndarray, w: np.ndarray):
    x = np.ascontiguousarray(x, dtype=np.float32).reshape(R, K)
    w = np.ascontiguousarray(w, dtype=np.float32)
    fp8 = ml_dtypes.float8_e4m3fn
    xh8 = x.astype(fp8)
    xl8 = (x - xh8.astype(np.float32)).astype(fp8)

    # rows -> [core, mt, mp], K -> [g, slot, kp]; tile layout [g, kp, mt, slot, mp]
    def xform(a):
        ar = a.reshape(NCORES, MT, P, G, 2, P)
        return np.ascontiguousarray(ar.transpose(0, 3, 5, 1, 4, 2))

    xh_t = xform(xh8)
    xl_t = xform(xl8)[:, :NLO]
    # w [g, slot, kp, nch, nn] -> [nch, kp, g, slot, nn], e4m3 (exact ternary)
    wr = np.ascontiguousarray(
        w.reshape(G, 2, P, NCHUNKS, NCH).transpose(3, 2, 0, 1, 4).astype(fp8)
    )
    return [{"xh": xh_t[c], "xl": xl_t[c], "w": wr} for c in range(NCORES)]


def _gather_output(results):
    y = np.stack([np.asarray(r["y"]) for r in results])  # [core, MT, P, N]
    return y.reshape(B, M, N).astype(np.float32)


def run(x: np.ndarray, w: np.ndarray, trace: bool = False):
    """Returns (y, BassKernelResults)."""
    nc = _get_program()
    in_maps = _prepare_in_maps(x, w)
    res = run_bass_kernel_spmd(
        nc, in_maps, core_ids=list(range(NCORES)), trace=trace
    )
    return _gather_output(res.results), res


def kernel(x: np.ndarray, w: np.ndarray) -> np.ndarray:
    y, _ = run(x, w, trace=False)
    return y
